# revision 2
# baseline (speedup 1.0000x reference)
"""DoReMi proxy-training loss on 8 Trainium2 NeuronCores.

Strategy (data-parallel over batch, per the sharding hint):
  - The only heavy tensor is sharded_logits [8, 1024, 32000] f32 (~1 GB).
    Core i streams batch row i ([1024, 32000], 131 MB) through SBUF once,
    computing per-token sum(exp(logits)) with a single fused ACT
    instruction per tile (Exp + accum_out row-reduction). This is the
    memory-roofline part; everything downstream is O(B*S) or O(N_DOMAINS)
    and is done on host in float64.
  - The label-logit gather (8192 elements) is host-side preprocessing:
    loss[b,s] = log(sum_v exp(logits[b,s,v])) - logits[b,s,label[b,s]].
    (logits ~ N(0,1), so the max-subtraction in log_softmax is not needed
    for fp32 range.)
"""

import numpy as np

B, S, V = 8, 1024, 32000
N_DOMAINS = 8
STEP_SIZE = 1.0
SMOOTHING = 1e-3

P = 128                 # SBUF partitions
NBLK = S // P           # 8 token blocks per core
VCHUNK = 8000           # vocab chunk per tile

# The DMA stream is the roofline; the kernel tail is the final chunk's ACT.
# Taper the last block's chunks so the last ACT after the final DMA packet
# is short (~1 us instead of ~6.7 us for a full 8000-wide chunk).
CHUNKS_MAIN = [VCHUNK] * (V // VCHUNK)
CHUNKS_LAST = [8000, 8000, 8000, 4000, 2000, 1000, 1000]
assert sum(CHUNKS_MAIN) == V and sum(CHUNKS_LAST) == V

_CACHED_NC = None


def _build_nc():
    import concourse.bacc as bacc
    import concourse.mybir as mybir
    import concourse.tile as tile

    nc = bacc.Bacc(enable_partition_id=False)
    x = nc.dram_tensor("x", [S, V], mybir.dt.float32, kind="ExternalInput")
    y = nc.dram_tensor("y", [P, NBLK], mybir.dt.float32, kind="ExternalOutput")
    with tile.TileContext(nc) as tc:
        with (
            tc.tile_pool(name="data", bufs=4) as dpool,
            tc.tile_pool(name="stats", bufs=2) as spool,
            tc.tile_pool(name="out", bufs=1) as opool,
        ):
            ytile = opool.tile([P, NBLK], mybir.dt.float32)
            for blk in range(NBLK):
                chunks = CHUNKS_LAST if blk == NBLK - 1 else CHUNKS_MAIN
                stats = spool.tile([P, len(chunks)], mybir.dt.float32)
                off = 0
                for ci, w in enumerate(chunks):
                    t = dpool.tile([P, w], mybir.dt.float32)
                    nc.sync.dma_start(
                        out=t[:],
                        in_=x[blk * P:(blk + 1) * P, off:off + w],
                    )
                    nc.scalar.activation(
                        t[:], t[:], mybir.ActivationFunctionType.Exp,
                        accum_out=stats[:, ci:ci + 1],
                    )
                    off += w
                nc.vector.reduce_sum(
                    ytile[:, blk:blk + 1], stats[:], axis=mybir.AxisListType.X
                )
            nc.sync.dma_start(out=y[:], in_=ytile[:])
    nc.compile()
    return nc


def _device_sumexp(logits: np.ndarray) -> np.ndarray:
    """logits [B, S, V] f32 -> sum(exp(logits), axis=-1) [B, S] f32 on 8 cores."""
    global _CACHED_NC
    from concourse.bass_utils import run_bass_kernel_spmd

    if _CACHED_NC is None:
        _CACHED_NC = _build_nc()
    in_maps = [{"x": np.ascontiguousarray(logits[i])} for i in range(B)]
    res = run_bass_kernel_spmd(_CACHED_NC, in_maps, core_ids=list(range(B)))
    out = np.empty((B, S), dtype=np.float32)
    for i in range(B):
        # y[p, blk] holds token blk*P + p
        out[i] = res.results[i]["y"].T.reshape(S)
    return out


def kernel(**inputs):
    logits = np.asarray(inputs["sharded_logits"], dtype=np.float32)
    label_ids = np.asarray(inputs["label_ids"]).astype(np.int64)
    label_mask = np.asarray(inputs["label_mask"]).astype(bool)
    domain_idxs = np.asarray(inputs["domain_idxs"]).astype(np.int64)
    ref_losses = np.asarray(inputs["ref_losses"], dtype=np.float32)
    domain_weights = np.asarray(inputs["domain_weights"], dtype=np.float32)

    sumexp = _device_sumexp(logits)  # [B, S] f32

    label_logit = np.take_along_axis(logits, label_ids[..., None], axis=-1)[..., 0]
    loss = np.log(sumexp.astype(np.float64)) - label_logit.astype(np.float64)

    mask = label_mask.astype(np.float64)
    ce_loss = (loss * mask).sum() / mask.sum()

    excess = np.maximum(loss - ref_losses.astype(np.float64), 0.0)
    per_sample = excess.sum(axis=-1)  # [B]

    domain_losses = np.zeros(N_DOMAINS, dtype=np.float64)
    np.add.at(domain_losses, domain_idxs, per_sample)
    samples = np.zeros(N_DOMAINS, dtype=np.float64)
    np.add.at(samples, domain_idxs, 1.0)

    with np.errstate(invalid="ignore", divide="ignore"):
        normalized = domain_losses / (samples * S)
    normalized = np.where(np.isnan(normalized), 0.0, normalized)

    log_w = np.log(domain_weights.astype(np.float64)) + STEP_SIZE * normalized
    m = log_w.max()
    log_w = log_w - (m + np.log(np.exp(log_w - m).sum()))
    train_w = (1.0 - SMOOTHING) * np.exp(log_w) + SMOOTHING / N_DOMAINS

    dro_loss = (train_w * normalized).sum()

    return (
        np.float32(ce_loss),
        np.float32(dro_loss),
        normalized.astype(np.float32),
        train_w.astype(np.float32),
        samples.astype(np.int32),
    )


# revision 4
# speedup vs baseline: 1.0158x; 1.0158x over previous
"""DoReMi proxy-training loss on 8 Trainium2 NeuronCores.

Strategy (data-parallel over batch, per the sharding hint):
  - The only heavy tensor is sharded_logits [8, 1024, 32000] f32 (~1 GB).
    Core i streams batch row i ([1024, 32000], 131 MB) through SBUF once,
    computing per-token sum(exp(logits)) with a single fused ACT
    instruction per tile (Exp + accum_out row-reduction). This is the
    memory-roofline part; everything downstream is O(B*S) or O(N_DOMAINS)
    and is done on host in float64.
  - The label-logit gather (8192 elements) is host-side preprocessing:
    loss[b,s] = log(sum_v exp(logits[b,s,v])) - logits[b,s,label[b,s]].
    (logits ~ N(0,1), so the max-subtraction in log_softmax is not needed
    for fp32 range.)
"""

import numpy as np

B, S, V = 8, 1024, 32000
N_DOMAINS = 8
STEP_SIZE = 1.0
SMOOTHING = 1e-3

P = 128                 # SBUF partitions
NBLK = S // P           # 8 token blocks per core
VCHUNK = 8000           # vocab chunk per tile

# The DMA stream is the roofline; the kernel tail is the final chunk's ACT.
# Taper the last block's chunks so the last ACT after the final DMA packet
# is short (~0.7 us instead of ~6.7 us for a full 8000-wide chunk).
CHUNKS_MAIN = [VCHUNK] * (V // VCHUNK)
CHUNKS_LAST = [8000, 8000, 8000, 4000, 2000, 1500, 500]
assert sum(CHUNKS_MAIN) == V and sum(CHUNKS_LAST) == V
# per-block partial-sum column layout in the device output
COL_OF_BLOCK = []
_c = 0
for _b in range(NBLK):
    _n = len(CHUNKS_LAST if _b == NBLK - 1 else CHUNKS_MAIN)
    COL_OF_BLOCK.append((_c, _c + _n))
    _c += _n
NCOLS = _c

_CACHED_NC = None


def _build_nc():
    import concourse.bacc as bacc
    import concourse.mybir as mybir
    import concourse.tile as tile

    nc = bacc.Bacc(enable_partition_id=False)
    x = nc.dram_tensor("x", [S, V], mybir.dt.float32, kind="ExternalInput")
    y = nc.dram_tensor("y", [P, NCOLS], mybir.dt.float32, kind="ExternalOutput")
    with tile.TileContext(nc) as tc:
        with (
            tc.tile_pool(name="data", bufs=4) as dpool,
            tc.tile_pool(name="out", bufs=1) as opool,
        ):
            # ACT accumulates each chunk's per-token sum straight into its own
            # column; the host adds the 4-7 partials per block. No Vector hop.
            ytile = opool.tile([P, NCOLS], mybir.dt.float32)
            col = 0
            for blk in range(NBLK):
                chunks = CHUNKS_LAST if blk == NBLK - 1 else CHUNKS_MAIN
                off = 0
                for w in chunks:
                    t = dpool.tile([P, w], mybir.dt.float32)
                    nc.sync.dma_start(
                        out=t[:],
                        in_=x[blk * P:(blk + 1) * P, off:off + w],
                    )
                    nc.scalar.activation(
                        t[:], t[:], mybir.ActivationFunctionType.Exp,
                        accum_out=ytile[:, col:col + 1],
                    )
                    off += w
                    col += 1
                if blk == NBLK - 2:
                    # ship blocks 0..6 partials early, off the critical path
                    nc.sync.dma_start(
                        out=y[:, :col], in_=ytile[:, :col]
                    )
            nc.sync.dma_start(
                out=y[:, COL_OF_BLOCK[-1][0]:],
                in_=ytile[:, COL_OF_BLOCK[-1][0]:],
            )
    nc.compile()
    return nc


def _device_sumexp(logits: np.ndarray) -> np.ndarray:
    """logits [B, S, V] f32 -> sum(exp(logits), axis=-1) [B, S] f32 on 8 cores."""
    global _CACHED_NC
    from concourse.bass_utils import run_bass_kernel_spmd

    if _CACHED_NC is None:
        _CACHED_NC = _build_nc()
    in_maps = [{"x": np.ascontiguousarray(logits[i])} for i in range(B)]
    res = run_bass_kernel_spmd(_CACHED_NC, in_maps, core_ids=list(range(B)))
    out = np.empty((B, S), dtype=np.float32)
    for i in range(B):
        ycols = res.results[i]["y"]  # [P, NCOLS]; y[p, col] partial for token blk*P+p
        for blk, (c0, c1) in enumerate(COL_OF_BLOCK):
            out[i, blk * P:(blk + 1) * P] = ycols[:, c0:c1].sum(axis=1)
    return out


def kernel(**inputs):
    logits = np.asarray(inputs["sharded_logits"], dtype=np.float32)
    label_ids = np.asarray(inputs["label_ids"]).astype(np.int64)
    label_mask = np.asarray(inputs["label_mask"]).astype(bool)
    domain_idxs = np.asarray(inputs["domain_idxs"]).astype(np.int64)
    ref_losses = np.asarray(inputs["ref_losses"], dtype=np.float32)
    domain_weights = np.asarray(inputs["domain_weights"], dtype=np.float32)

    sumexp = _device_sumexp(logits)  # [B, S] f32

    label_logit = np.take_along_axis(logits, label_ids[..., None], axis=-1)[..., 0]
    loss = np.log(sumexp.astype(np.float64)) - label_logit.astype(np.float64)

    mask = label_mask.astype(np.float64)
    ce_loss = (loss * mask).sum() / mask.sum()

    excess = np.maximum(loss - ref_losses.astype(np.float64), 0.0)
    per_sample = excess.sum(axis=-1)  # [B]

    domain_losses = np.zeros(N_DOMAINS, dtype=np.float64)
    np.add.at(domain_losses, domain_idxs, per_sample)
    samples = np.zeros(N_DOMAINS, dtype=np.float64)
    np.add.at(samples, domain_idxs, 1.0)

    with np.errstate(invalid="ignore", divide="ignore"):
        normalized = domain_losses / (samples * S)
    normalized = np.where(np.isnan(normalized), 0.0, normalized)

    log_w = np.log(domain_weights.astype(np.float64)) + STEP_SIZE * normalized
    m = log_w.max()
    log_w = log_w - (m + np.log(np.exp(log_w - m).sum()))
    train_w = (1.0 - SMOOTHING) * np.exp(log_w) + SMOOTHING / N_DOMAINS

    dro_loss = (train_w * normalized).sum()

    return (
        np.float32(ce_loss),
        np.float32(dro_loss),
        normalized.astype(np.float32),
        train_w.astype(np.float32),
        samples.astype(np.int32),
    )


# revision 5
# speedup vs baseline: 1.1039x; 1.0867x over previous
"""DoReMi proxy-training loss on 8 Trainium2 NeuronCores.

Strategy (data-parallel over batch, per the sharding hint):
  - The only heavy tensor is sharded_logits [8, 1024, 32000] f32 (~1 GB).
    Core i streams batch row i ([1024, 32000], 131 MB) through SBUF once,
    computing per-token sum(exp(logits)) with a single fused ACT
    instruction per tile (Exp + accum_out row-reduction). This is the
    memory-roofline part; everything downstream is O(B*S) or O(N_DOMAINS)
    and is done on host in float64.
  - The label-logit gather (8192 elements) is host-side preprocessing:
    loss[b,s] = log(sum_v exp(logits[b,s,v])) - logits[b,s,label[b,s]].
    (logits ~ N(0,1), so the max-subtraction in log_softmax is not needed
    for fp32 range.)
"""

import numpy as np

B, S, V = 8, 1024, 32000
N_DOMAINS = 8
STEP_SIZE = 1.0
SMOOTHING = 1e-3

P = 128                 # SBUF partitions
NBLK = S // P           # 8 token blocks per core
VCHUNK = 8000           # vocab chunk per tile

# The DMA stream is the roofline; the kernel tail is the final chunk's ACT.
# Taper the last block's chunks so the last ACT after the final DMA packet
# is short (~0.7 us instead of ~6.7 us for a full 8000-wide chunk).
CHUNKS_MAIN = [VCHUNK] * (V // VCHUNK)
CHUNKS_LAST = [8000, 8000, 8000, 4000, 2000, 1500, 500]
assert sum(CHUNKS_MAIN) == V and sum(CHUNKS_LAST) == V
# per-block partial-sum column layout in the device output
COL_OF_BLOCK = []
_c = 0
for _b in range(NBLK):
    _n = len(CHUNKS_LAST if _b == NBLK - 1 else CHUNKS_MAIN)
    COL_OF_BLOCK.append((_c, _c + _n))
    _c += _n
NCOLS = _c

_CACHED_NC = None


def _build_nc():
    import concourse.bacc as bacc
    import concourse.mybir as mybir
    import concourse.tile as tile

    nc = bacc.Bacc(enable_partition_id=False)
    x = nc.dram_tensor("x", [S, V], mybir.dt.float32, kind="ExternalInput")
    y = nc.dram_tensor("y", [P, NCOLS], mybir.dt.float32, kind="ExternalOutput")
    with tile.TileContext(nc) as tc:
        with (
            tc.tile_pool(name="data", bufs=4) as dpool,
            tc.tile_pool(name="out", bufs=1) as opool,
        ):
            # ACT accumulates each chunk's per-token sum straight into its own
            # column; the host adds the 4-7 partials per block. No Vector hop.
            ytile = opool.tile([P, NCOLS], mybir.dt.float32)
            col = 0
            for blk in range(NBLK):
                chunks = CHUNKS_LAST if blk == NBLK - 1 else CHUNKS_MAIN
                off = 0
                for w in chunks:
                    t = dpool.tile([P, w], mybir.dt.float32)
                    nc.sync.dma_start(
                        out=t[:],
                        in_=x[blk * P:(blk + 1) * P, off:off + w],
                    )
                    nc.scalar.activation(
                        t[:], t[:], mybir.ActivationFunctionType.Exp,
                        accum_out=ytile[:, col:col + 1],
                    )
                    off += w
                    col += 1
                if blk == NBLK - 2:
                    # ship blocks 0..6 partials early, off the critical path;
                    # issue from the Scalar engine (HWDGE) so the Sync queue
                    # never stalls waiting on ACT sems.
                    nc.scalar.dma_start(
                        out=y[:, :col], in_=ytile[:, :col]
                    )
            # final columns also from Scalar: same engine as the last ACT, so
            # the issue needs no cross-engine sem hop.
            nc.scalar.dma_start(
                out=y[:, COL_OF_BLOCK[-1][0]:],
                in_=ytile[:, COL_OF_BLOCK[-1][0]:],
            )
    nc.compile()
    return nc


def _device_sumexp(logits: np.ndarray) -> np.ndarray:
    """logits [B, S, V] f32 -> sum(exp(logits), axis=-1) [B, S] f32 on 8 cores."""
    global _CACHED_NC
    from concourse.bass_utils import run_bass_kernel_spmd

    if _CACHED_NC is None:
        _CACHED_NC = _build_nc()
    in_maps = [{"x": np.ascontiguousarray(logits[i])} for i in range(B)]
    res = run_bass_kernel_spmd(_CACHED_NC, in_maps, core_ids=list(range(B)))
    out = np.empty((B, S), dtype=np.float32)
    for i in range(B):
        ycols = res.results[i]["y"]  # [P, NCOLS]; y[p, col] partial for token blk*P+p
        for blk, (c0, c1) in enumerate(COL_OF_BLOCK):
            out[i, blk * P:(blk + 1) * P] = ycols[:, c0:c1].sum(axis=1)
    return out


def kernel(**inputs):
    logits = np.asarray(inputs["sharded_logits"], dtype=np.float32)
    label_ids = np.asarray(inputs["label_ids"]).astype(np.int64)
    label_mask = np.asarray(inputs["label_mask"]).astype(bool)
    domain_idxs = np.asarray(inputs["domain_idxs"]).astype(np.int64)
    ref_losses = np.asarray(inputs["ref_losses"], dtype=np.float32)
    domain_weights = np.asarray(inputs["domain_weights"], dtype=np.float32)

    sumexp = _device_sumexp(logits)  # [B, S] f32

    label_logit = np.take_along_axis(logits, label_ids[..., None], axis=-1)[..., 0]
    loss = np.log(sumexp.astype(np.float64)) - label_logit.astype(np.float64)

    mask = label_mask.astype(np.float64)
    ce_loss = (loss * mask).sum() / mask.sum()

    excess = np.maximum(loss - ref_losses.astype(np.float64), 0.0)
    per_sample = excess.sum(axis=-1)  # [B]

    domain_losses = np.zeros(N_DOMAINS, dtype=np.float64)
    np.add.at(domain_losses, domain_idxs, per_sample)
    samples = np.zeros(N_DOMAINS, dtype=np.float64)
    np.add.at(samples, domain_idxs, 1.0)

    with np.errstate(invalid="ignore", divide="ignore"):
        normalized = domain_losses / (samples * S)
    normalized = np.where(np.isnan(normalized), 0.0, normalized)

    log_w = np.log(domain_weights.astype(np.float64)) + STEP_SIZE * normalized
    m = log_w.max()
    log_w = log_w - (m + np.log(np.exp(log_w - m).sum()))
    train_w = (1.0 - SMOOTHING) * np.exp(log_w) + SMOOTHING / N_DOMAINS

    dro_loss = (train_w * normalized).sum()

    return (
        np.float32(ce_loss),
        np.float32(dro_loss),
        normalized.astype(np.float32),
        train_w.astype(np.float32),
        samples.astype(np.int32),
    )
